# revision 13
# baseline (speedup 1.0000x reference)
"""CMOEBlock Trainium2 kernel (8-core SPMD, single NEFF, on-device AllReduce pool).

Sharding: 72 patches (2 batches x 6x6 grid of 64x64 patches), 9 per core.
Cores 0-3 hold batch 0, cores 4-7 batch 1. Expert routing weights are
aggregated per patch on the host (sum_k v[n,k] * w[k]) and folded with the
LayerNorm affine, beta/gamma residual scales and biases, so the device only
runs dense per-patch convs:

  phase A: LN1 stats (PE matmuls) -> s,t rows -> z = x*s -> conv1 (K=66 aug
           matmul incl. -r*t and bias) -> 3x3 conv2 (9 taps + bias matmul,
           per-patch zero padding) -> SimpleGate -> keep (bf16) -> pool
           partial sums -> spill keep to DRAM
  AllReduce pooled over replica groups [[0..3],[4..7]]
  phase B: fused (conv3 o diag(pool) o sca) via tiny on-device matmul ->
           y = comb(keep) + I*inp (identity-add matmul) -> LN2 -> conv4 ->
           SimpleGate -> conv5 (+gamma) + I*y -> out

Matmuls run in float32r (~tf32 precision, full PE rate at N=512).
"""
import os
import sys

sys.path.insert(0, "/opt/trn_rl_repo")

import numpy as np
import ml_dtypes

import concourse.bass as bass
import concourse.tile as tile
from concourse import bacc, mybir
from concourse.bass_utils import run_bass_kernel_spmd

F32 = mybir.dt.float32
F32R = mybir.dt.float32r
BF16 = mybir.dt.bfloat16
AF = mybir.ActivationFunctionType
ALU = mybir.AluOpType

B, C, H, W, P, NEXP = 2, 64, 384, 384, 64, 5
DW = 2 * C
HP = H // P               # 6
NPATCH = B * HP * HP      # 72
NCORE = 8
PPC = NPATCH // NCORE     # 9 patches per core
PIX = P * P               # 4096
CW = 512                  # pixel chunk width (1 PSUM bank)
NCH = PIX // CW           # 8
PADW = P + 2              # 66
PADN = PADW * PADW        # 4356
EPS = 1e-6
POOL_SCALE = 1.0 / float(H * W)

_LOG_FN = getattr(AF, "Ln", None) or getattr(AF, "Log", None)


def _bcast_src(row_tile_ap, row, width):
    """AP reading one SBUF row replicated across `width`... partitions? No:
    source AP [1 partition] x [0-step 64] x [width] for a partition-broadcast DMA."""
    t = row_tile_ap.tensor
    pitch = row_tile_ap.ap[-1][0] * row_tile_ap.shape[-1] if False else None
    return bass.AP(t, row * width, [[1, 1], [0, 64], [1, width]])


def _build_nc():
    nc = bacc.Bacc("TRN2", target_bir_lowering=False, debug=False,
                   num_devices=NCORE)

    # ---- DRAM I/O ----
    xin = nc.dram_tensor("xin", [PPC, C, PIX], F32R, kind="ExternalInput")
    wz1 = nc.dram_tensor("wz1", [PPC, 66, DW], F32R, kind="ExternalInput")
    w2t = nc.dram_tensor("w2t", [PPC, DW, 9 * DW], F32R, kind="ExternalInput")
    b2 = nc.dram_tensor("b2", [PPC, 1, DW], F32R, kind="ExternalInput")
    wcms = nc.dram_tensor("wcms", [PPC, C, 65], BF16, kind="ExternalInput")
    wcmp = nc.dram_tensor("wcmp", [PPC, C, C], BF16, kind="ExternalInput")
    wcmb = nc.dram_tensor("wcmb", [PPC, 1, C], BF16, kind="ExternalInput")
    wz4 = nc.dram_tensor("wz4", [PPC, 66, DW], F32R, kind="ExternalInput")
    w5t = nc.dram_tensor("w5t", [PPC, 65, C], BF16, kind="ExternalInput")
    wstat = nc.dram_tensor("wstat", [128, 512], F32R, kind="ExternalInput")
    ident = nc.dram_tensor("ident", [C, C], F32R, kind="ExternalInput")
    e64 = nc.dram_tensor("e64", [1, 65], BF16, kind="ExternalInput")
    onesr = nc.dram_tensor("onesr", [1, PIX], F32R, kind="ExternalInput")
    onesb = nc.dram_tensor("onesb", [1, PIX], BF16, kind="ExternalInput")
    out_d = nc.dram_tensor("out", [PPC, C, PIX], F32, kind="ExternalOutput")

    keepsp = nc.dram_tensor("keepsp", [PPC, C, PIX], BF16)
    tb1 = nc.dram_tensor("tb1", [PPC, PIX], F32R)
    tb2 = nc.dram_tensor("tb2", [PPC, PIX], F32R)
    sb1 = nc.dram_tensor("sb1", [PPC, PIX], F32)
    sb2 = nc.dram_tensor("sb2", [PPC, PIX], F32)
    pool_in = nc.dram_tensor("pool_in", [128, 1], F32)
    pool_out = nc.dram_tensor("pool_out", [128, 1], F32, addr_space="Shared")
    bsel = nc.dram_tensor("bsel", [128, C], F32R, kind="ExternalInput")
    bselT = nc.dram_tensor("bselT", [C, 128], F32R, kind="ExternalInput")

    with tile.TileContext(nc) as tc:
        with tc.tile_pool(name="consts", bufs=1) as p_const, \
             tc.tile_pool(name="xsq", bufs=2) as p_xsq, \
             tc.tile_pool(name="zt", bufs=2) as p_z, \
             tc.tile_pool(name="big", bufs=2) as p_big, \
             tc.tile_pool(name="sbc", bufs=1) as p_sbc, \
             tc.tile_pool(name="keep", bufs=2) as p_keep, \
             tc.tile_pool(name="sg2", bufs=1) as p_sg2, \
             tc.tile_pool(name="outp", bufs=1) as p_out, \
             tc.tile_pool(name="w2p", bufs=2) as p_w2, \
             tc.tile_pool(name="wsm", bufs=2) as p_wsm, \
             tc.tile_pool(name="rows", bufs=1) as p_row, \
             tc.tile_pool(name="sgt", bufs=2) as p_sgt, \
             tc.tile_pool(name="junk", bufs=1) as p_junk, \
             tc.tile_pool(name="ps_stat", bufs=2, space="PSUM") as ps_stat, \
             tc.tile_pool(name="ps_c1", bufs=2, space="PSUM") as ps_c1, \
             tc.tile_pool(name="ps_c2", bufs=2, space="PSUM") as ps_c2, \
             tc.tile_pool(name="ps_cmb", bufs=1, space="PSUM") as ps_cmb:

            # ---- constants loaded once ----
            wstat_t = p_const.tile([128, 512], F32R, tag="wstat")
            nc.sync.dma_start(wstat_t[:], wstat[:])
            ident_t = p_const.tile([C, C], F32R, tag="ident")
            nc.sync.dma_start(ident_t[:], ident[:])
            e64_t = p_const.tile([1, 65], BF16, tag="e64")
            nc.sync.dma_start(e64_t[:], e64[:])
            ones512 = p_const.tile([1, CW], F32R, tag="ones512")
            nc.sync.dma_start(ones512[:], onesr[:][0:1, 0:CW])
            poolacc = p_const.tile([C, 80], F32, tag="poolacc")
            nc.vector.memset(poolacc[:], 0.0)
            eps8 = p_const.tile([8, 1], F32, tag="eps8")
            nc.vector.memset(eps8[:], EPS)

            xin_a, out_a = xin[:], out_d[:]
            keepsp_a, tb1_a, tb2_a = keepsp[:], tb1[:], tb2[:]

            def ln_rows(xsq_t, trow_bounce_ap, srow_bounce_ap, tag_sfx):
                """LN stats for one patch from xsq tile ([0:64]=x, [64:128]=x^2).
                Returns (srow_tile, trow written to DRAM bounce)."""
                stat = ps_stat.tile([64, CW], F32, tag="stat")
                for c in range(NCH):
                    nc.tensor.matmul(
                        stat[:], wstat_t[:, 64 * c:64 * c + 64],
                        xsq_t[:, CW * c:CW * (c + 1)],
                        start=(c == 0), stop=(c == NCH - 1))
                musq = p_row.tile([8, CW], F32, tag="musq")
                nc.scalar.square(musq[:], stat[0:8, :])
                vrow = p_row.tile([8, CW], F32, tag="vrow")
                nc.vector.tensor_sub(vrow[:], stat[32:40, :], musq[:])
                lnv = p_row.tile([8, CW], F32, tag="lnv")
                nc.scalar.activation(lnv[:], vrow[:], _LOG_FN, bias=eps8[:])
                srow = p_row.tile([8, CW], F32, tag="srow")
                nc.scalar.activation(srow[:], lnv[:], AF.Exp, scale=-0.5)
                trow = p_row.tile([8, CW], F32R, tag="trow")
                nc.vector.tensor_tensor(trow[:], stat[0:8, :], srow[:], ALU.mult)
                nc.sync.dma_start(trow_bounce_ap, trow[:])
                nc.sync.dma_start(srow_bounce_ap, srow[:])
                return srow

            def make_z(x_rows_f32, sb_handle, j, trow_bounce_ap, z_ones_src):
                """z tile [66, PIX]: rows 0-63 = x*s, row 64 = t, row 65 = ones."""
                z = p_z.tile([66, PIX], F32R, tag="z")
                nc.sync.dma_start(z[64:65, :], trow_bounce_ap)
                nc.sync.dma_start(z[65:66, :], z_ones_src)
                sbc = p_sbc.tile([C, PIX], F32, tag="sbc")
                src = bass.AP(sb_handle[:].tensor, j * PIX, [[0, C], [1, PIX]])
                nc.sync.dma_start(sbc[:], src)
                nc.vector.tensor_tensor(z[0:C, :], x_rows_f32, sbc[:], ALU.mult)
                return z

            # ======================= PHASE A =======================
            for j in range(PPC):
                xsq = p_xsq.tile([128, PIX], F32R, tag="xsq")
                nc.sync.dma_start(xsq[0:C, :], xin_a[j])
                w2tile = p_w2.tile([DW, 9 * DW], F32R, tag="w2")
                nc.sync.dma_start(w2tile[:], w2t[:][j])
                wz1t = p_wsm.tile([66, DW], F32R, tag="wz")
                nc.sync.dma_start(wz1t[:], wz1[:][j])
                b2t = p_wsm.tile([1, DW], F32R, tag="b2")
                nc.sync.dma_start(b2t[:], b2[:][j])

                nc.scalar.square(xsq[C:128, :], xsq[0:C, :])
                srow = ln_rows(xsq, tb1_a[j], sb1[:][j], "a")
                z1 = make_z(xsq[0:C, :].bitcast(F32), sb1, j, tb1_a[j], onesr[:])

                # conv1 -> padded conv2 input
                pad = p_big.tile([128, PADN], F32R, tag="big")
                padh = pad[:].tensor
                padf = pad[:].bitcast(F32)
                nc.vector.memset(padf[:, 0:PADW], 0.0)
                nc.vector.memset(padf[:, PADN - PADW:PADN], 0.0)
                nc.vector.memset(
                    bass.AP(padh, PADW, [[PADN, 128], [PADW, P], [1, 1]]).bitcast(F32), 0.0)
                nc.vector.memset(
                    bass.AP(padh, PADW + P + 1, [[PADN, 128], [PADW, P], [1, 1]]).bitcast(F32), 0.0)
                for c in range(NCH):
                    c1 = ps_c1.tile([128, CW], F32, tag="c1")
                    nc.tensor.matmul(c1[:], wz1t[:], z1[:, CW * c:CW * (c + 1)],
                                     start=True, stop=True)
                    dst = bass.AP(padh, (1 + 8 * c) * PADW + 1,
                                  [[PADN, 128], [PADW, 8], [1, P]])
                    src = c1[:].rearrange("p (a b) -> p a b", a=8)
                    if c % 2 == 0:
                        nc.scalar.copy(dst, src)
                    else:
                        nc.vector.tensor_copy(dst, src)

                # conv2 (9 taps + bias) -> SG -> keep
                keep = p_keep.tile([65, PIX], BF16, tag="keep")
                nc.sync.dma_start(keep[64:65, :], onesb[:])
                for c in range(NCH):
                    c2 = ps_c2.tile([128, CW], F32, tag="c2")
                    c2v = c2[:].rearrange("p (a b) -> p a b", a=8)
                    for t in range(9):
                        dy, dx = t // 3, t % 3
                        rhs = bass.AP(padh, (8 * c + dy) * PADW + dx,
                                      [[PADN, 128], [PADW, 8], [1, P]])
                        nc.tensor.matmul(c2v, w2tile[:, DW * t:DW * (t + 1)], rhs,
                                         start=(t == 0), stop=False)
                    nc.tensor.matmul(c2[:], b2t[:], ones512[:], start=False, stop=True)
                    sgt = p_sgt.tile([C, CW], F32, tag="sgt")
                    nc.scalar.copy(sgt[:], c2[C:128, :])
                    nc.vector.tensor_tensor(keep[0:C, CW * c:CW * (c + 1)],
                                            c2[0:C, :], sgt[:], ALU.mult)
                    # pool partial sums (gpsimd, accum over free dim)
                    junk = p_junk.tile([C, CW], BF16, tag="junk")
                    nc.vector.tensor_scalar(
                        junk[:], keep[0:C, CW * c:CW * (c + 1)], 1.0, 0.0,
                        ALU.mult, ALU.add,
                        accum_out=poolacc[:, 8 * j + c:8 * j + c + 1])
                nc.sync.dma_start(keepsp_a[j], keep[0:C, :])

            # ============== pooled: reduce + AllReduce ==============
            bsel_t = p_const.tile([128, C], F32R, tag="bsel")
            nc.sync.dma_start(bsel_t[:], bsel[:])
            bselT_t = p_const.tile([C, 128], F32R, tag="bselT")
            nc.sync.dma_start(bselT_t[:], bselT[:])
            poolsum = p_const.tile([C, 8], F32R, tag="poolsum")
            nc.vector.memset(poolsum[:].bitcast(F32), 0.0)
            with nc.allow_low_precision(reason="f32r == f32 width"):
                nc.vector.reduce_sum(poolsum[:, 0:1], poolacc[:, 0:8 * PPC],
                                     axis=mybir.AxisListType.X)
            # scatter my batch's partial into global [128] slot layout
            scat = ps_cmb.tile([128, 8], F32, tag="cmb")
            nc.tensor.matmul(scat[:], bselT_t[:], poolsum[:], start=True, stop=True)
            poolfull = p_const.tile([128, 1], F32, tag="poolfull")
            nc.scalar.copy(poolfull[:], scat[:, 0:1])
            nc.sync.dma_start(pool_in[:], poolfull[:])
            nc.gpsimd.collective_compute(
                "AllReduce", ALU.add,
                replica_groups=[list(range(NCORE))],
                ins=[pool_in[:]], outs=[pool_out[:]])
            pout_f = p_const.tile([128, 1], F32, tag="pout_f")
            nc.sync.dma_start(pout_f[:], pool_out[:])
            pout_r = p_const.tile([128, 8], F32R, tag="pout_r")
            nc.vector.memset(pout_r[:].bitcast(F32), 0.0)
            nc.vector.tensor_copy(pout_r[:, 0:1], pout_f[:])
            sel = ps_cmb.tile([C, 8], F32, tag="cmb")
            nc.tensor.matmul(sel[:], bsel_t[:], pout_r[:], start=True, stop=True)
            pooledsc = p_const.tile([C, 1], F32, tag="pooledsc")
            nc.scalar.activation(pooledsc[:], sel[:, 0:1], AF.Copy, scale=POOL_SCALE)

            # ======================= PHASE B =======================
            for j in range(PPC):
                keepb = p_keep.tile([65, PIX], BF16, tag="keep")
                nc.sync.dma_start(keepb[0:C, :], keepsp_a[j])
                nc.sync.dma_start(keepb[64:65, :], onesb[:])
                xb = p_big.tile([C, PIX], F32R, tag="big")
                nc.sync.dma_start(xb[:], xin_a[j])
                wcms_t = p_wsm.tile([C, 65], BF16, tag="wcms")
                nc.sync.dma_start(wcms_t[:], wcms[:][j])
                wcmp_t = p_wsm.tile([C, C], BF16, tag="wcmp")
                nc.sync.dma_start(wcmp_t[:], wcmp[:][j])
                wcmb_t = p_wsm.tile([1, C], BF16, tag="wcmb")
                nc.sync.dma_start(wcmb_t[:], wcmb[:][j])
                wz4t = p_wsm.tile([66, DW], F32R, tag="wz")
                nc.sync.dma_start(wz4t[:], wz4[:][j])
                w5t_t = p_wsm.tile([65, C], BF16, tag="w5")
                nc.sync.dma_start(w5t_t[:], w5t[:][j])

                # fused conv3 o diag(pool) o sca weights
                rhscm = p_wsm.tile([C, C], BF16, tag="rhscm")
                nc.vector.tensor_scalar(rhscm[:], wcmp_t[:], pooledsc[:], None,
                                        ALU.mult)
                cmb = ps_cmb.tile([65, C], F32, tag="cmb")
                nc.tensor.matmul(cmb[:], wcms_t[:], rhscm[:], start=True, stop=False)
                nc.tensor.matmul(cmb[:], e64_t[:], wcmb_t[:], start=False, stop=True)
                lhscomb = p_wsm.tile([65, C], BF16, tag="lhscomb")
                nc.scalar.copy(lhscomb[:], cmb[:])

                # y = comb(keep) + I*inp ; ysq rows 0-63 = y, 64-127 = y^2
                ysq = p_xsq.tile([128, PIX], F32R, tag="xsq")
                for c in range(NCH):
                    yps = ps_c1.tile([C, CW], F32, tag="c1")
                    nc.tensor.matmul(yps[:], lhscomb[:],
                                     keepb[:, CW * c:CW * (c + 1)],
                                     start=True, stop=False)
                    nc.tensor.matmul(yps[:], ident_t[:],
                                     xb[:, CW * c:CW * (c + 1)],
                                     start=False, stop=True)
                    nc.scalar.copy(ysq[0:C, CW * c:CW * (c + 1)], yps[:])
                    nc.vector.tensor_tensor(
                        ysq[C:128, CW * c:CW * (c + 1)], yps[:],
                        ysq[0:C, CW * c:CW * (c + 1)].bitcast(F32), ALU.mult)

                srow2 = ln_rows(ysq, tb2_a[j], sb2[:][j], "b")
                z2 = make_z(ysq[0:C, :].bitcast(F32), sb2, j, tb2_a[j], onesr[:])

                # conv4 -> SG2 -> sg2 tile
                sg2 = p_sg2.tile([65, PIX], BF16, tag="sg2")
                nc.sync.dma_start(sg2[64:65, :], onesb[:])
                for c in range(NCH):
                    c4 = ps_c2.tile([128, CW], F32, tag="c2")
                    nc.tensor.matmul(c4[:], wz4t[:], z2[:, CW * c:CW * (c + 1)],
                                     start=True, stop=True)
                    sgt = p_sgt.tile([C, CW], F32, tag="sgt")
                    nc.scalar.copy(sgt[:], c4[C:128, :])
                    nc.vector.tensor_tensor(sg2[0:C, CW * c:CW * (c + 1)],
                                            c4[0:C, :], sgt[:], ALU.mult)

                # out = y + gamma*conv5(sg2)  (gamma/bias folded in w5t)
                outt = p_out.tile([C, PIX], F32, tag="outt")
                for c in range(NCH):
                    ops = ps_c1.tile([C, CW], F32, tag="c1")
                    nc.tensor.matmul(ops[:], w5t_t[:],
                                     sg2[:, CW * c:CW * (c + 1)],
                                     start=True, stop=False)
                    nc.tensor.matmul(ops[:], ident_t[:],
                                     ysq[0:C, CW * c:CW * (c + 1)],
                                     start=False, stop=True)
                    if c % 2 == 0:
                        nc.scalar.copy(outt[:, CW * c:CW * (c + 1)], ops[:])
                    else:
                        nc.vector.tensor_copy(outt[:, CW * c:CW * (c + 1)], ops[:])
                nc.sync.dma_start(out_a[j], outt[:])

    nc.compile()
    return nc


_NC = None


def _get_nc():
    global _NC
    if _NC is None:
        _NC = _build_nc()
    return _NC


def _host_fold(inputs):
    """Aggregate expert weights per patch and fold LN/beta/gamma. Returns
    per-patch arrays indexed by global patch id n = b*36 + hy*6 + hx."""
    f8 = np.float32
    v = np.asarray(inputs["v"], f8)
    vr = v.transpose(0, 2, 3, 1).reshape(NPATCH, NEXP)          # [n, k]

    def agg(w):
        w = np.asarray(w, f8)
        return np.einsum("nk,koihw->noihw", vr, w, optimize=True)[..., 0, 0]

    def aggb(b):
        return vr @ np.asarray(b, f8)

    n1w = np.asarray(inputs["n1w"], f8)
    n1b = np.asarray(inputs["n1b"], f8)
    n2w = np.asarray(inputs["n2w"], f8)
    n2b = np.asarray(inputs["n2b"], f8)
    beta = np.asarray(inputs["beta"], f8).reshape(C)
    gamma = np.asarray(inputs["gamma"], f8).reshape(C)

    aw1, ab1 = agg(inputs["w1"]), aggb(inputs["b1"])            # [n,128,64],[n,128]
    aw3, ab3 = agg(inputs["w3"]), aggb(inputs["b3"])            # [n,64,64]
    awsca, absca = agg(inputs["wsca"]), aggb(inputs["bsca"])
    aw4, ab4 = agg(inputs["w4"]), aggb(inputs["b4"])
    aw5, ab5 = agg(inputs["w5"]), aggb(inputs["b5"])

    aw2 = np.asarray(inputs["w2"], f8)
    aw2 = np.einsum("nk,koihw->noihw", vr, aw2, optimize=True)   # [n,128,128,3,3]
    ab2 = aggb(inputs["b2"])

    def ln_fold(aw, ab, gw, gb):
        ag = aw * gw[None, None, :]
        r = ag.sum(axis=2)
        q = np.einsum("noc,c->no", aw, gb) + ab
        return np.concatenate(
            [ag.transpose(0, 2, 1), -r[:, None, :], q[:, None, :]], axis=1)

    wz1 = ln_fold(aw1, ab1, n1w, n1b)                            # [n, 66, 128]
    wz4 = ln_fold(aw4, ab4, n2w, n2b)

    w2t = aw2.transpose(0, 2, 3, 4, 1).reshape(NPATCH, DW, 9 * DW)
    b2r = ab2[:, None, :]                                        # [n, 1, 128]

    wcms = np.concatenate([awsca, absca[:, :, None]], axis=2)    # [n, 64, 65]
    aw3b = beta[None, :, None] * aw3
    wcmp = aw3b.transpose(0, 2, 1)                               # [n, m, o]
    wcmb = (beta * ab3)[:, None, :]                              # [n, 1, 64]

    aw5g = gamma[None, :, None] * aw5
    w5tt = np.concatenate(
        [aw5g.transpose(0, 2, 1), (gamma * ab5)[:, None, :]], axis=1)  # [n,65,64]

    wstat = np.zeros((128, 512), np.float32)
    for c in range(NCH):
        wstat[0:C, 64 * c + c] = 1.0 / C
        wstat[C:128, 64 * c + 32 + c] = 1.0 / C

    e64a = np.zeros((1, 65), np.float32)
    e64a[0, 64] = 1.0

    f32 = np.float32
    bf = ml_dtypes.bfloat16
    return dict(
        wz1=wz1.astype(f32), w2t=w2t.astype(f32), b2=b2r.astype(f32),
        wcms=wcms.astype(bf), wcmp=wcmp.astype(bf), wcmb=wcmb.astype(bf),
        wz4=wz4.astype(f32), w5t=w5tt.astype(bf),
        wstat=wstat, ident=np.eye(C, dtype=f32), e64=e64a.astype(bf),
        onesr=np.ones((1, PIX), f32), onesb=np.ones((1, PIX), bf),
    )


def kernel(**inputs):
    assert int(np.asarray(inputs["patch_size"])) == P
    inp = np.ascontiguousarray(np.asarray(inputs["inp"], np.float32))
    folded = _host_fold(inputs)

    # patch extraction: n = b*36 + hy*6 + hx
    xpatch = inp.reshape(B, C, HP, P, HP, P).transpose(0, 2, 4, 1, 3, 5)
    xpatch = np.ascontiguousarray(xpatch.reshape(NPATCH, C, PIX))

    shared = {k: folded[k] for k in ("wstat", "ident", "e64", "onesr", "onesb")}
    in_maps = []
    for core in range(NCORE):
        sl = slice(core * PPC, (core + 1) * PPC)
        m = dict(shared)
        m["xin"] = xpatch[sl]
        S = np.zeros((128, C), np.float32)
        S[C * (core // 4):C * (core // 4) + C, :] = np.eye(C, dtype=np.float32)
        m["bsel"] = S
        m["bselT"] = np.ascontiguousarray(S.T)
        for k in ("wz1", "w2t", "b2", "wcms", "wcmp", "wcmb", "wz4", "w5t"):
            m[k] = np.ascontiguousarray(folded[k][sl])
        in_maps.append(m)

    nc = _get_nc()
    import time as _time
    t0 = _time.time()
    res = run_bass_kernel_spmd(nc, in_maps, list(range(NCORE)))
    kernel.last_exec_ns = res.exec_time_ns
    kernel.last_wall_ns = int((_time.time() - t0) * 1e9)

    outp = np.empty((NPATCH, C, PIX), np.float32)
    for core in range(NCORE):
        outp[core * PPC:(core + 1) * PPC] = res.results[core]["out"]
    outf = outp.reshape(B, HP, HP, C, P, P).transpose(0, 3, 1, 4, 2, 5)
    outf = np.ascontiguousarray(outf.reshape(B, C, H, W))
    return outf, np.asarray(inputs["v"], np.float32)


kernel.last_exec_ns = None
kernel.last_wall_ns = None


# revision 16
# speedup vs baseline: 1.0685x; 1.0685x over previous
"""CMOEBlock Trainium2 kernel (8-core SPMD, single NEFF, on-device AllReduce pool).

Sharding: 72 patches (2 batches x 6x6 grid of 64x64 patches), 9 per core.
Cores 0-3 hold batch 0, cores 4-7 batch 1. Expert routing weights are
aggregated per patch on the host (sum_k v[n,k] * w[k]) and folded with the
LayerNorm affine, beta/gamma residual scales and biases, so the device only
runs dense per-patch convs:

  phase A: LN1 stats (PE matmuls) -> s,t rows -> z = x*s -> conv1 (K=66 aug
           matmul incl. -r*t and bias) -> 3x3 conv2 (9 taps + bias matmul,
           per-patch zero padding) -> SimpleGate -> keep (bf16) -> pool
           partial sums -> spill keep to DRAM
  AllReduce pooled over replica groups [[0..3],[4..7]]
  phase B: fused (conv3 o diag(pool) o sca) via tiny on-device matmul ->
           y = comb(keep) + I*inp (identity-add matmul) -> LN2 -> conv4 ->
           SimpleGate -> conv5 (+gamma) + I*y -> out

Matmuls run in float32r (~tf32 precision, full PE rate at N=512).
"""
import os
import sys

sys.path.insert(0, "/opt/trn_rl_repo")

import numpy as np
import ml_dtypes

import concourse.bass as bass
import concourse.tile as tile
from concourse import bacc, mybir
from concourse.bass_utils import run_bass_kernel_spmd

F32 = mybir.dt.float32
F32R = mybir.dt.float32r
BF16 = mybir.dt.bfloat16
AF = mybir.ActivationFunctionType
ALU = mybir.AluOpType

B, C, H, W, P, NEXP = 2, 64, 384, 384, 64, 5
DW = 2 * C
HP = H // P               # 6
NPATCH = B * HP * HP      # 72
NCORE = 8
PPC = NPATCH // NCORE     # 9 patches per core
PIX = P * P               # 4096
CW = 512                  # pixel chunk width (1 PSUM bank)
NCH = PIX // CW           # 8
PADW = P + 2              # 66
PADN = PADW * PADW        # 4356
EPS = 1e-6
POOL_SCALE = 1.0 / float(H * W)

_LOG_FN = getattr(AF, "Ln", None) or getattr(AF, "Log", None)


def _bcast_src(row_tile_ap, row, width):
    """AP reading one SBUF row replicated across `width`... partitions? No:
    source AP [1 partition] x [0-step 64] x [width] for a partition-broadcast DMA."""
    t = row_tile_ap.tensor
    pitch = row_tile_ap.ap[-1][0] * row_tile_ap.shape[-1] if False else None
    return bass.AP(t, row * width, [[1, 1], [0, 64], [1, width]])


def _build_nc():
    nc = bacc.Bacc("TRN2", target_bir_lowering=False, debug=False,
                   num_devices=NCORE)

    # ---- DRAM I/O ----
    xin = nc.dram_tensor("xin", [PPC, C, PIX], F32R, kind="ExternalInput")
    wz1 = nc.dram_tensor("wz1", [PPC, 66, DW], F32R, kind="ExternalInput")
    w2t = nc.dram_tensor("w2t", [PPC, DW, 9 * DW], F32R, kind="ExternalInput")
    b2 = nc.dram_tensor("b2", [PPC, 1, DW], F32R, kind="ExternalInput")
    wcms = nc.dram_tensor("wcms", [PPC, C, 65], BF16, kind="ExternalInput")
    wcmp = nc.dram_tensor("wcmp", [PPC, C, C], BF16, kind="ExternalInput")
    wcmb = nc.dram_tensor("wcmb", [PPC, 1, C], BF16, kind="ExternalInput")
    wz4 = nc.dram_tensor("wz4", [PPC, 66, DW], F32R, kind="ExternalInput")
    w5t = nc.dram_tensor("w5t", [PPC, 65, C], BF16, kind="ExternalInput")
    wstat = nc.dram_tensor("wstat", [128, 512], F32R, kind="ExternalInput")
    ident = nc.dram_tensor("ident", [C, C], F32R, kind="ExternalInput")
    e64 = nc.dram_tensor("e64", [1, 65], BF16, kind="ExternalInput")
    onesr = nc.dram_tensor("onesr", [1, PIX], F32R, kind="ExternalInput")
    onesb = nc.dram_tensor("onesb", [1, PIX], BF16, kind="ExternalInput")
    out_d = nc.dram_tensor("out", [PPC, C, PIX], F32, kind="ExternalOutput")

    keepsp = nc.dram_tensor("keepsp", [PPC, C, PIX], BF16)
    tb1 = nc.dram_tensor("tb1", [PPC, PIX], F32R)
    tb2 = nc.dram_tensor("tb2", [PPC, PIX], F32R)
    sb1 = nc.dram_tensor("sb1", [PPC, PIX], F32)
    sb2 = nc.dram_tensor("sb2", [PPC, PIX], F32)
    pool_in = nc.dram_tensor("pool_in", [128, 1], F32)
    pool_out = nc.dram_tensor("pool_out", [128, 1], F32, addr_space="Shared")
    bsel = nc.dram_tensor("bsel", [128, C], F32R, kind="ExternalInput")
    bselT = nc.dram_tensor("bselT", [C, 128], F32R, kind="ExternalInput")

    with tile.TileContext(nc) as tc:
        with tc.tile_pool(name="consts", bufs=1) as p_const, \
             tc.tile_pool(name="xsq", bufs=2) as p_xsq, \
             tc.tile_pool(name="zt", bufs=2) as p_z, \
             tc.tile_pool(name="big", bufs=2) as p_big, \
             tc.tile_pool(name="sbc", bufs=1) as p_sbc, \
             tc.tile_pool(name="keep", bufs=2) as p_keep, \
             tc.tile_pool(name="sg2", bufs=1) as p_sg2, \
             tc.tile_pool(name="outp", bufs=1) as p_out, \
             tc.tile_pool(name="w2p", bufs=2) as p_w2, \
             tc.tile_pool(name="wsm", bufs=2) as p_wsm, \
             tc.tile_pool(name="rows", bufs=1) as p_row, \
             tc.tile_pool(name="sgt", bufs=2) as p_sgt, \
             tc.tile_pool(name="junk", bufs=1) as p_junk, \
             tc.tile_pool(name="ps_stat", bufs=2, space="PSUM") as ps_stat, \
             tc.tile_pool(name="ps_c1", bufs=2, space="PSUM") as ps_c1, \
             tc.tile_pool(name="ps_c2", bufs=2, space="PSUM") as ps_c2, \
             tc.tile_pool(name="ps_cmb", bufs=1, space="PSUM") as ps_cmb:

            # ---- constants loaded once ----
            wstat_t = p_const.tile([128, 512], F32R, tag="wstat")
            nc.sync.dma_start(wstat_t[:], wstat[:])
            ident_t = p_const.tile([C, C], F32R, tag="ident")
            nc.sync.dma_start(ident_t[:], ident[:])
            e64_t = p_const.tile([1, 65], BF16, tag="e64")
            nc.sync.dma_start(e64_t[:], e64[:])
            ones512 = p_const.tile([1, CW], F32R, tag="ones512")
            nc.sync.dma_start(ones512[:], onesr[:][0:1, 0:CW])
            poolacc = p_const.tile([C, 80], F32, tag="poolacc")
            nc.vector.memset(poolacc[:], 0.0)
            eps8 = p_const.tile([8, 1], F32, tag="eps8")
            nc.vector.memset(eps8[:], EPS)

            xin_a, out_a = xin[:], out_d[:]
            keepsp_a, tb1_a, tb2_a = keepsp[:], tb1[:], tb2[:]

            def ln_rows(xsq_t, trow_bounce_ap, srow_bounce_ap, tag_sfx):
                """LN stats for one patch from xsq tile ([0:64]=x, [64:128]=x^2).
                Returns (srow_tile, trow written to DRAM bounce)."""
                stat = ps_stat.tile([64, CW], F32, tag="stat")
                for c in range(NCH):
                    nc.tensor.matmul(
                        stat[:], wstat_t[:, 64 * c:64 * c + 64],
                        xsq_t[:, CW * c:CW * (c + 1)],
                        start=(c == 0), stop=(c == NCH - 1))
                musq = p_row.tile([8, CW], F32, tag="musq")
                nc.scalar.square(musq[:], stat[0:8, :])
                vrow = p_row.tile([8, CW], F32, tag="vrow")
                nc.vector.tensor_sub(vrow[:], stat[32:40, :], musq[:])
                lnv = p_row.tile([8, CW], F32, tag="lnv")
                nc.scalar.activation(lnv[:], vrow[:], _LOG_FN, bias=eps8[:])
                srow = p_row.tile([8, CW], F32, tag="srow")
                nc.scalar.activation(srow[:], lnv[:], AF.Exp, scale=-0.5)
                trow = p_row.tile([8, CW], F32R, tag="trow")
                nc.vector.tensor_tensor(trow[:], stat[0:8, :], srow[:], ALU.mult)
                nc.sync.dma_start(trow_bounce_ap, trow[:])
                nc.sync.dma_start(srow_bounce_ap, srow[:])
                return srow

            def make_z(x_rows_f32, sb_handle, j, trow_bounce_ap, z_ones_src):
                """z tile [66, PIX]: rows 0-63 = x*s, row 64 = t, row 65 = ones."""
                z = p_z.tile([66, PIX], F32R, tag="z")
                nc.sync.dma_start(z[64:65, :], trow_bounce_ap)
                nc.sync.dma_start(z[65:66, :], z_ones_src)
                sbc = p_sbc.tile([C, PIX], F32, tag="sbc")
                src = bass.AP(sb_handle[:].tensor, j * PIX, [[0, C], [1, PIX]])
                nc.sync.dma_start(sbc[:], src)
                nc.vector.tensor_tensor(z[0:C, :], x_rows_f32, sbc[:], ALU.mult)
                return z

            # ======================= PHASE A =======================
            for j in range(PPC):
                xsq = p_xsq.tile([128, PIX], F32R, tag="xsq")
                nc.sync.dma_start(xsq[0:C, :], xin_a[j])
                w2tile = p_w2.tile([DW, 9 * DW], F32R, tag="w2")
                nc.sync.dma_start(w2tile[:], w2t[:][j])
                wz1t = p_wsm.tile([66, DW], F32R, tag="wz")
                nc.sync.dma_start(wz1t[:], wz1[:][j])
                b2t = p_wsm.tile([1, DW], F32R, tag="b2")
                nc.sync.dma_start(b2t[:], b2[:][j])

                nc.scalar.square(xsq[C:128, :], xsq[0:C, :])
                srow = ln_rows(xsq, tb1_a[j], sb1[:][j], "a")
                z1 = make_z(xsq[0:C, :].bitcast(F32), sb1, j, tb1_a[j], onesr[:])

                # conv1 -> padded conv2 input
                pad = p_big.tile([128, PADN], F32R, tag="big")
                padh = pad[:].tensor
                padf = pad[:].bitcast(F32)
                nc.vector.memset(padf[:, 0:PADW], 0.0)
                nc.vector.memset(padf[:, PADN - PADW:PADN], 0.0)
                nc.vector.memset(
                    bass.AP(padh, PADW, [[PADN, 128], [PADW, P], [1, 1]]).bitcast(F32), 0.0)
                nc.vector.memset(
                    bass.AP(padh, PADW + P + 1, [[PADN, 128], [PADW, P], [1, 1]]).bitcast(F32), 0.0)
                for c in range(NCH):
                    c1 = ps_c1.tile([128, CW], F32, tag="c1")
                    nc.tensor.matmul(c1[:], wz1t[:], z1[:, CW * c:CW * (c + 1)],
                                     start=True, stop=True)
                    dst = bass.AP(padh, (1 + 8 * c) * PADW + 1,
                                  [[PADN, 128], [PADW, 8], [1, P]])
                    src = c1[:].rearrange("p (a b) -> p a b", a=8)
                    if c % 2 == 0:
                        nc.scalar.copy(dst, src)
                    else:
                        nc.vector.tensor_copy(dst, src)

                # conv2 (9 taps + bias) -> SG -> keep
                keep = p_keep.tile([65, PIX], BF16, tag="keep")
                nc.sync.dma_start(keep[64:65, :], onesb[:])
                for c in range(NCH):
                    c2 = ps_c2.tile([128, CW], F32, tag="c2")
                    c2v = c2[:].rearrange("p (a b) -> p a b", a=8)
                    for t in range(9):
                        dy, dx = t // 3, t % 3
                        rhs = bass.AP(padh, (8 * c + dy) * PADW + dx,
                                      [[PADN, 128], [PADW, 8], [1, P]])
                        nc.tensor.matmul(c2v, w2tile[:, DW * t:DW * (t + 1)], rhs,
                                         start=(t == 0), stop=False)
                    nc.tensor.matmul(c2[:], b2t[:], ones512[:], start=False, stop=True)
                    sgt = p_sgt.tile([C, CW], F32, tag="sgt")
                    nc.scalar.copy(sgt[:], c2[C:128, :])
                    nc.vector.tensor_tensor(keep[0:C, CW * c:CW * (c + 1)],
                                            c2[0:C, :], sgt[:], ALU.mult)
                    # pool partial sums (gpsimd, accum over free dim)
                    junk = p_junk.tile([C, CW], BF16, tag="junk")
                    nc.vector.tensor_scalar(
                        junk[:], keep[0:C, CW * c:CW * (c + 1)], 1.0, 0.0,
                        ALU.mult, ALU.add,
                        accum_out=poolacc[:, 8 * j + c:8 * j + c + 1])
                nc.sync.dma_start(keepsp_a[j], keep[0:C, :])

            # ============== pooled: reduce + AllReduce ==============
            bsel_t = p_const.tile([128, C], F32R, tag="bsel")
            nc.sync.dma_start(bsel_t[:], bsel[:])
            bselT_t = p_const.tile([C, 128], F32R, tag="bselT")
            nc.sync.dma_start(bselT_t[:], bselT[:])
            poolsum = p_const.tile([C, 8], F32R, tag="poolsum")
            nc.vector.memset(poolsum[:].bitcast(F32), 0.0)
            with nc.allow_low_precision(reason="f32r == f32 width"):
                nc.vector.reduce_sum(poolsum[:, 0:1], poolacc[:, 0:8 * PPC],
                                     axis=mybir.AxisListType.X)
            # scatter my batch's partial into global [128] slot layout
            scat = ps_cmb.tile([128, 8], F32, tag="cmb")
            nc.tensor.matmul(scat[:], bselT_t[:], poolsum[:], start=True, stop=True)
            poolfull = p_const.tile([128, 1], F32, tag="poolfull")
            nc.scalar.copy(poolfull[:], scat[:, 0:1])
            nc.sync.dma_start(pool_in[:], poolfull[:])
            nc.gpsimd.collective_compute(
                "AllReduce", ALU.add,
                replica_groups=[list(range(NCORE))],
                ins=[pool_in[:]], outs=[pool_out[:]])
            pout_f = p_const.tile([128, 1], F32, tag="pout_f")
            nc.sync.dma_start(pout_f[:], pool_out[:])
            pout_r = p_const.tile([128, 8], F32R, tag="pout_r")
            nc.vector.memset(pout_r[:].bitcast(F32), 0.0)
            nc.vector.tensor_copy(pout_r[:, 0:1], pout_f[:])
            sel = ps_cmb.tile([C, 8], F32, tag="cmb")
            nc.tensor.matmul(sel[:], bsel_t[:], pout_r[:], start=True, stop=True)
            pooledsc = p_const.tile([C, 1], F32, tag="pooledsc")
            nc.scalar.activation(pooledsc[:], sel[:, 0:1], AF.Copy, scale=POOL_SCALE)

            # ======================= PHASE B =======================
            for j in range(PPC):
                keepb = p_keep.tile([65, PIX], BF16, tag="keep")
                nc.sync.dma_start(keepb[0:C, :], keepsp_a[j])
                nc.sync.dma_start(keepb[64:65, :], onesb[:])
                xb = p_big.tile([C, PIX], F32R, tag="big")
                nc.sync.dma_start(xb[:], xin_a[j])
                wcms_t = p_wsm.tile([C, 65], BF16, tag="wcms")
                nc.sync.dma_start(wcms_t[:], wcms[:][j])
                wcmp_t = p_wsm.tile([C, C], BF16, tag="wcmp")
                nc.sync.dma_start(wcmp_t[:], wcmp[:][j])
                wcmb_t = p_wsm.tile([1, C], BF16, tag="wcmb")
                nc.sync.dma_start(wcmb_t[:], wcmb[:][j])
                wz4t = p_wsm.tile([66, DW], F32R, tag="wz")
                nc.sync.dma_start(wz4t[:], wz4[:][j])
                w5t_t = p_wsm.tile([65, C], BF16, tag="w5")
                nc.sync.dma_start(w5t_t[:], w5t[:][j])

                # fused conv3 o diag(pool) o sca weights
                rhscm = p_wsm.tile([C, C], BF16, tag="rhscm")
                nc.vector.tensor_scalar(rhscm[:], wcmp_t[:], pooledsc[:], None,
                                        ALU.mult)
                cmb = ps_cmb.tile([65, C], F32, tag="cmb")
                nc.tensor.matmul(cmb[:], wcms_t[:], rhscm[:], start=True, stop=False)
                nc.tensor.matmul(cmb[:], e64_t[:], wcmb_t[:], start=False, stop=True)
                lhscomb = p_wsm.tile([65, C], BF16, tag="lhscomb")
                nc.scalar.copy(lhscomb[:], cmb[:])

                # y = comb(keep) + I*inp ; ysq rows 0-63 = y, 64-127 = y^2
                ysq = p_xsq.tile([128, PIX], F32R, tag="xsq")
                for c in range(NCH):
                    yps = ps_c1.tile([C, CW], F32, tag="c1")
                    nc.tensor.matmul(yps[:], lhscomb[:],
                                     keepb[:, CW * c:CW * (c + 1)],
                                     start=True, stop=False)
                    nc.tensor.matmul(yps[:], ident_t[:],
                                     xb[:, CW * c:CW * (c + 1)],
                                     start=False, stop=True)
                    nc.scalar.copy(ysq[0:C, CW * c:CW * (c + 1)], yps[:])
                    nc.vector.tensor_tensor(
                        ysq[C:128, CW * c:CW * (c + 1)], yps[:],
                        ysq[0:C, CW * c:CW * (c + 1)].bitcast(F32), ALU.mult)

                srow2 = ln_rows(ysq, tb2_a[j], sb2[:][j], "b")
                z2 = make_z(ysq[0:C, :].bitcast(F32), sb2, j, tb2_a[j], onesr[:])

                # conv4 -> SG2 -> sg2 tile
                sg2 = p_sg2.tile([65, PIX], BF16, tag="sg2")
                nc.sync.dma_start(sg2[64:65, :], onesb[:])
                for c in range(NCH):
                    c4 = ps_c2.tile([128, CW], F32, tag="c2")
                    nc.tensor.matmul(c4[:], wz4t[:], z2[:, CW * c:CW * (c + 1)],
                                     start=True, stop=True)
                    sgt = p_sgt.tile([C, CW], F32, tag="sgt")
                    nc.scalar.copy(sgt[:], c4[C:128, :])
                    nc.vector.tensor_tensor(sg2[0:C, CW * c:CW * (c + 1)],
                                            c4[0:C, :], sgt[:], ALU.mult)

                # out = y + gamma*conv5(sg2)  (gamma/bias folded in w5t)
                outt = p_out.tile([C, PIX], F32, tag="outt")
                for c in range(NCH):
                    ops = ps_c1.tile([C, CW], F32, tag="c1")
                    nc.tensor.matmul(ops[:], w5t_t[:],
                                     sg2[:, CW * c:CW * (c + 1)],
                                     start=True, stop=False)
                    nc.tensor.matmul(ops[:], ident_t[:],
                                     ysq[0:C, CW * c:CW * (c + 1)],
                                     start=False, stop=True)
                    if c % 2 == 0:
                        nc.scalar.copy(outt[:, CW * c:CW * (c + 1)], ops[:])
                    else:
                        nc.vector.tensor_copy(outt[:, CW * c:CW * (c + 1)], ops[:])
                nc.sync.dma_start(out_a[j], outt[:])

    nc.compile()
    return nc


_NC = None


def _get_nc():
    global _NC
    if _NC is None:
        _NC = _build_nc()
    return _NC


def _host_fold(inputs):
    """Aggregate expert weights per patch and fold LN/beta/gamma. Returns
    per-patch arrays indexed by global patch id n = b*36 + hy*6 + hx."""
    f8 = np.float32
    v = np.asarray(inputs["v"], f8)
    vr = v.transpose(0, 2, 3, 1).reshape(NPATCH, NEXP)          # [n, k]

    def agg(w):
        w = np.asarray(w, f8)
        return np.einsum("nk,koihw->noihw", vr, w, optimize=True)[..., 0, 0]

    def aggb(b):
        return vr @ np.asarray(b, f8)

    n1w = np.asarray(inputs["n1w"], f8)
    n1b = np.asarray(inputs["n1b"], f8)
    n2w = np.asarray(inputs["n2w"], f8)
    n2b = np.asarray(inputs["n2b"], f8)
    beta = np.asarray(inputs["beta"], f8).reshape(C)
    gamma = np.asarray(inputs["gamma"], f8).reshape(C)

    aw1, ab1 = agg(inputs["w1"]), aggb(inputs["b1"])            # [n,128,64],[n,128]
    aw3, ab3 = agg(inputs["w3"]), aggb(inputs["b3"])            # [n,64,64]
    awsca, absca = agg(inputs["wsca"]), aggb(inputs["bsca"])
    aw4, ab4 = agg(inputs["w4"]), aggb(inputs["b4"])
    aw5, ab5 = agg(inputs["w5"]), aggb(inputs["b5"])

    aw2 = np.asarray(inputs["w2"], f8)
    aw2 = np.einsum("nk,koihw->noihw", vr, aw2, optimize=True)   # [n,128,128,3,3]
    ab2 = aggb(inputs["b2"])

    def ln_fold(aw, ab, gw, gb):
        ag = aw * gw[None, None, :]
        r = ag.sum(axis=2)
        q = np.einsum("noc,c->no", aw, gb) + ab
        return np.concatenate(
            [ag.transpose(0, 2, 1), -r[:, None, :], q[:, None, :]], axis=1)

    wz1 = ln_fold(aw1, ab1, n1w, n1b)                            # [n, 66, 128]
    wz4 = ln_fold(aw4, ab4, n2w, n2b)

    w2t = aw2.transpose(0, 2, 3, 4, 1).reshape(NPATCH, DW, 9 * DW)
    b2r = ab2[:, None, :]                                        # [n, 1, 128]

    wcms = np.concatenate([awsca, absca[:, :, None]], axis=2)    # [n, 64, 65]
    aw3b = beta[None, :, None] * aw3
    wcmp = aw3b.transpose(0, 2, 1)                               # [n, m, o]
    wcmb = (beta * ab3)[:, None, :]                              # [n, 1, 64]

    aw5g = gamma[None, :, None] * aw5
    w5tt = np.concatenate(
        [aw5g.transpose(0, 2, 1), (gamma * ab5)[:, None, :]], axis=1)  # [n,65,64]

    wstat = np.zeros((128, 512), np.float32)
    for c in range(NCH):
        wstat[0:C, 64 * c + c] = 1.0 / C
        wstat[C:128, 64 * c + 32 + c] = 1.0 / C

    e64a = np.zeros((1, 65), np.float32)
    e64a[0, 64] = 1.0

    f32 = np.float32
    bf = ml_dtypes.bfloat16
    return dict(
        wz1=wz1.astype(f32), w2t=w2t.astype(f32), b2=b2r.astype(f32),
        wcms=wcms.astype(bf), wcmp=wcmp.astype(bf), wcmb=wcmb.astype(bf),
        wz4=wz4.astype(f32), w5t=w5tt.astype(bf),
        wstat=wstat, ident=np.eye(C, dtype=f32), e64=e64a.astype(bf),
        onesr=np.ones((1, PIX), f32), onesb=np.ones((1, PIX), bf),
    )


def kernel(**inputs):
    assert int(np.asarray(inputs["patch_size"])) == P
    inp = np.ascontiguousarray(np.asarray(inputs["inp"], np.float32))
    folded = _host_fold(inputs)

    # patch extraction: n = b*36 + hy*6 + hx
    xpatch = inp.reshape(B, C, HP, P, HP, P).transpose(0, 2, 4, 1, 3, 5)
    xpatch = np.ascontiguousarray(xpatch.reshape(NPATCH, C, PIX))

    shared = {k: folded[k] for k in ("wstat", "ident", "e64", "onesr", "onesb")}
    in_maps = []
    for core in range(NCORE):
        sl = slice(core * PPC, (core + 1) * PPC)
        m = dict(shared)
        m["xin"] = xpatch[sl]
        S = np.zeros((128, C), np.float32)
        S[C * (core // 4):C * (core // 4) + C, :] = np.eye(C, dtype=np.float32)
        m["bsel"] = S
        m["bselT"] = np.ascontiguousarray(S.T)
        for k in ("wz1", "w2t", "b2", "wcms", "wcmp", "wcmb", "wz4", "w5t"):
            m[k] = np.ascontiguousarray(folded[k][sl])
        in_maps.append(m)

    nc = _get_nc()
    import time as _time
    t0 = _time.time()
    res = run_bass_kernel_spmd(nc, in_maps, list(range(NCORE)))
    kernel.last_exec_ns = res.exec_time_ns
    kernel.last_wall_ns = int((_time.time() - t0) * 1e9)

    outp = np.empty((NPATCH, C, PIX), np.float32)
    for core in range(NCORE):
        outp[core * PPC:(core + 1) * PPC] = res.results[core]["out"]
    outf = outp.reshape(B, HP, HP, C, P, P).transpose(0, 3, 1, 4, 2, 5)
    outf = np.ascontiguousarray(outf.reshape(B, C, H, W))
    return outf, np.asarray(inputs["v"], np.float32)


kernel.last_exec_ns = None
kernel.last_wall_ns = None
